# revision 21
# baseline (speedup 1.0000x reference)
"""EquivariantAttention Trainium2 kernel.

Reference computation (B=4, S=512, D=512, H=8, DH=64):
    qkv = x @ W_qkv                      -> q, k, v  (b, s, h, dh)
    geo_w = geometric_features @ W_geo   -> (b, h, i, j)
    pos_w = positional_encodings @ W_pos -> (h, i, j)
    scores = q k^T / sqrt(dh) + geo_w + pos_w
    attn   = softmax_j(scores)            (mask is all-ones -> no-op)
    out    = (attn @ v) @ W_out

Sharding: the positional_encodings table dominates HBM traffic, so the query
dim i is sharded across the 8 cores (64 rows each).  Every core computes full
k/v (cheap) and its own i-slice of the output; the host concats.

Within a core the scores live TRANSPOSED - j on partitions, (h, i) in the
free dim - because pos_w can only leave the tensor engine as out[M=j, N=h]
with d on the contraction partitions.  The host stages positional_encodings
pre-transposed to (d, i, j) and pre-cast to fp8-e3m4 (halving HBM bytes
again vs bf16; W_pos is staged x48 in e3m4 and the 1/48 is folded into the
score-bias add), so every tile lands partition-correct straight off a plain
HWDGE DMA.  geo_w is computed ON the tensor engine by contracting over
K=(c,i)=128 against a block-diagonal weight tile (wgeo[c,h] * I64) and lands
directly in the scores psum accumulation group, so the per-(b,h) DVE
broadcast chain and the gT transposes of the old scheme disappear and each
score tile needs just one DVE add (which also folds the 1/48) and one exp.
Softmax over j (= partitions) skips max-subtraction (scores are O(1)); the
denominator is a matmul against a ones-vector alongside the attn @ v matmuls.
"""

import numpy as np

B, S, D, H = 4, 512, 512, 8
DH = D // H            # 64
NCORES = 8
IS = S // NCORES       # 64  i-rows per core
T = B * S              # 2048 tokens
TI = B * IS            # 256 slice tokens
IGRP = 4               # i-rows per P-load DMA group
POS_WSCALE = 48.0      # W_pos staged x48 in e3m4; folded back in the bias add

_CACHE = {}


def _patch_ldw_opt():
    """Flip walrus's --enable-ldw-opt (hardcoded false in bass_utils) via a
    run_command shim.  The optimization merges/pipelines LDWEIGHTS; this
    kernel is weight-load bound (1024 pos-tile loads per iteration)."""
    import os
    import concourse.bass_utils as bu
    if os.environ.get("LDW_OPT", "0") != "1" or getattr(bu, "_ldw_patched", False):
        return
    orig = bu.run_command

    def patched(cmd, **kw):
        cmd = [c.replace("--enable-ldw-opt=false", "--enable-ldw-opt=true")
               if isinstance(c, str) else c for c in cmd]
        return orig(cmd, **kw)

    bu.run_command = patched
    bu._ldw_patched = True


def _build_program(iters=1, variant="full"):
    _patch_ldw_opt()
    import concourse.bacc as bacc
    import concourse.mybir as mybir
    import concourse.tile as tile
    from concourse.masks import make_identity

    f32 = mybir.dt.float32
    bf16 = mybir.dt.bfloat16
    f8 = mybir.dt.float8e3

    nc = bacc.Bacc(
        "TRN2",
        target_bir_lowering=False,
        debug=False,
        enable_asserts=False,
        num_devices=NCORES,
    )

    x_d = nc.dram_tensor("x", [D, T], bf16, kind="ExternalInput").ap()
    xs_d = nc.dram_tensor("x_slice", [D, TI], bf16, kind="ExternalInput").ap()
    # positional_encodings arrive host-pre-transposed to (d, i, j) so the
    # contraction dim d lands on SBUF partitions straight out of the DMA
    p_d = nc.dram_tensor("pos_enc", [4, D, IS, S // 4], f8,
                         kind="ExternalInput").ap()
    g_d = nc.dram_tensor("geo", [B, 2 * IS, S], bf16, kind="ExternalInput").ap()
    wqkv_d = nc.dram_tensor("w_qkv", [D, 3 * D], bf16, kind="ExternalInput").ap()
    wpos_d = nc.dram_tensor("w_pos", [D, H], f8, kind="ExternalInput").ap()
    wgeo_d = nc.dram_tensor("w_geo", [2, H], f32, kind="ExternalInput").ap()
    wout_d = nc.dram_tensor("w_out", [D, D], bf16, kind="ExternalInput").ap()
    out_d = nc.dram_tensor("out", [B, IS, D], f32, kind="ExternalOutput").ap()

    with tile.TileContext(nc) as tc:
        if iters == 1:
            _emit_iter(nc, tc, mybir, tile, make_identity,
                       x_d, xs_d, p_d, g_d, wqkv_d, wpos_d, wgeo_d, wout_d,
                       out_d, variant)
        else:
            # benchmark build: run the whole kernel `iters` times back-to-back
            # inside one NEFF so host dispatch overhead amortizes away.
            # hint_engines arms the branch prefetcher: the body far exceeds
            # one IRAM block, so the back-edge would otherwise stall ~4us on
            # the instruction fetch.
            with tc.For_i(0, iters, 1, hint_engines=mybir.ALL_ENGINES):
                _emit_iter(nc, tc, mybir, tile, make_identity,
                           x_d, xs_d, p_d, g_d, wqkv_d, wpos_d, wgeo_d,
                           wout_d, out_d, variant)

    nc.compile()
    return nc


def _emit_iter(nc, tc, mybir, tile, make_identity,
               x_d, xs_d, p_d, g_d, wqkv_d, wpos_d, wgeo_d, wout_d, out_d,
               variant="full"):
    from contextlib import ExitStack

    f32 = mybir.dt.float32
    bf16 = mybir.dt.bfloat16
    f8 = mybir.dt.float8e3
    AF = mybir.ActivationFunctionType
    ALU = mybir.AluOpType

    with ExitStack() as ctx:
        # ---------------- Phase 0: constants ----------------
        cp = ctx.enter_context(tc.tile_pool(name="consts", bufs=1))

        ident = cp.tile([128, 128], bf16, name="ident", tag="ident")
        make_identity(nc, ident)

        ones_col = cp.tile([128, 1], bf16, name="ones_col", tag="ones_col")
        nc.gpsimd.memset(ones_col, 1.0)

        wqkv_sb = []
        for dt_ in range(4):
            t_ = cp.tile([128, 3 * D], bf16, name=f"wqkv_{dt_}", tag=f"wqkv{dt_}")
            nc.scalar.dma_start(out=t_, in_=wqkv_d[dt_ * 128:(dt_ + 1) * 128, :])
            wqkv_sb.append(t_)

        # W_pos as (128, 4*8): [:, db*8:(db+1)*8] = rows db*128..db*128+127
        wpos_sb = cp.tile([128, 32], f8, name="wpos_sb", tag="wpos")
        nc.scalar.dma_start(
            out=wpos_sb.rearrange("p (a h) -> p a h", a=4),
            in_=wpos_d.rearrange("(a p) h -> p a h", p=128),
        )

        # W_geo broadcast to all partitions: (128, 16) f32, col c*8+h
        wgeo_flat = cp.tile([1, 16], f32, name="wgeo_flat", tag="wgf")
        nc.scalar.dma_start(
            out=wgeo_flat, in_=wgeo_d.rearrange("c h -> (c h)")[None, :])
        wgeo_fbf = cp.tile([1, 16], bf16, name="wgeo_fbf", tag="wgfb")
        nc.vector.tensor_copy(wgeo_fbf, wgeo_flat)
        wgeo_bc = cp.tile([128, 16], f32, name="wgeo_bc", tag="wgbc")
        ones_r128 = cp.tile([1, 128], bf16, name="ones_r128", tag="ones_r128")
        nc.gpsimd.memset(ones_r128, 1.0)
        with tc.tile_pool(name="bc_ps2", bufs=1, space="PSUM") as bc_pool:
            bc_ps = bc_pool.tile([128, 16], f32, name="bc_ps2", tag="bcps2")
            nc.tensor.matmul(bc_ps, ones_r128, wgeo_fbf, start=True, stop=True)
            nc.vector.tensor_copy(wgeo_bc, bc_ps)

        wout8_sb = []
        for h in range(H):
            t_ = cp.tile([DH, D], bf16, name=f"wout_{h}", tag=f"wout{h}")
            nc.scalar.dma_start(out=t_, in_=wout_d[h * DH:(h + 1) * DH, :])
            wout8_sb.append(t_)

        # geo features, natural layout: partition = (c, i) pair, free = j.
        # Host stages (B, 2, IS, S) which IS [(c i), j] per b contiguously.
        gsb = [cp.tile([2 * IS, S], bf16, name=f"gsb_{b}", tag=f"gsb{b}")
               for b in range(B)]
        for b in range(B):
            nc.scalar.dma_start(out=gsb[b], in_=g_d[b])

        # rhs_geo[(c,i), (h,i')] = wgeo[c,h] * I64[i,i']: contracting the
        # gsb tile against this on the PE computes geo_w[j, (h,i)] directly
        # into the scores psum - no per-h DVE chain, no gT transposes.
        rhs_geo = cp.tile([128, 512], bf16, name="rhs_geo", tag="rhsgeo")
        for c in range(2):
            for h in range(H):
                nc.vector.tensor_scalar(
                    rhs_geo[c * 64:(c + 1) * 64, h * 64:(h + 1) * 64],
                    ident[c * 64:(c + 1) * 64, c * 64:(c + 1) * 64],
                    wgeo_bc[c * 64:(c + 1) * 64, c * 8 + h:c * 8 + h + 1],
                    None, op0=ALU.mult)

        # ---------------- Phase 1: xT, xsT straight from host ------------
        # x arrives host-transposed (d, tokens) bf16: plain DMAs, no PE work
        xT_sb = [cp.tile([128, T], bf16, name=f"xT_{db}", tag=f"xT{db}")
                 for db in range(4)]
        xsT_sb = [cp.tile([128, TI], bf16, name=f"xsT_{db}", tag=f"xsT{db}")
                  for db in range(4)]
        for db in range(4):
            nc.scalar.dma_start(out=xT_sb[db], in_=x_d[db * 128:(db + 1) * 128, :])
            nc.scalar.dma_start(out=xsT_sb[db], in_=xs_d[db * 128:(db + 1) * 128, :])

        # ---------------- Phase 2 decls (work emitted inside phase 4) ----
        # k is stored in HEAD-PAIR tiles [128 (2 heads x dh), T]: one k-proj
        # matmul then serves two heads (x streams through the PE once per
        # pair instead of once per head).  The matching q tiles are 128-part
        # ZERO-PADDED (top half = even head, or bottom half = odd head, rest
        # zeros) so each score matmul contracts K=128 but the zero half
        # nulls the other head - no operand ever sits at base partition 64
        # (which hard-faults the exec unit, NRT_EXEC_UNIT_UNRECOVERABLE 101).
        HP = H // 2
        kT2_sb = [cp.tile([128, T], bf16, name=f"kT2_{hp}", tag=f"kT2{hp}")
                  for hp in range(HP)]
        v_sb = [cp.tile([128, D], bf16, name=f"v_{tt}", tag=f"v{tt}")
                for tt in range(T // 128)]
        q2z_a = [cp.tile([128, TI], bf16, name=f"q2a_{hp}", tag=f"q2a{hp}")
                 for hp in range(HP)]
        q2z_b = [cp.tile([128, TI], bf16, name=f"q2b_{hp}", tag=f"q2b{hp}")
                 for hp in range(HP)]
        for hp in range(HP):
            # full-tile memsets: partition-offset memsets fail the ISA check
            nc.gpsimd.memset(q2z_a[hp], 0.0)
            nc.gpsimd.memset(q2z_b[hp], 0.0)

        def _emit_kT2(proj_pool, hp, tch):
            ps = proj_pool.tile([128, 512], f32, name="ps_k", tag="pj")
            for dt_ in range(4):
                nc.tensor.matmul(
                    ps,
                    wqkv_sb[dt_][:, 512 + hp * 128: 512 + (hp + 1) * 128],
                    xT_sb[dt_][:, tch * 512:(tch + 1) * 512],
                    start=(dt_ == 0), stop=(dt_ == 3),
                )
            dst = kT2_sb[hp][:, tch * 512:(tch + 1) * 512]
            if (hp + tch) % 2 == 0:
                nc.vector.tensor_copy(dst, ps)
            else:
                nc.scalar.copy(dst, ps)

        def _emit_v(proj_pool, tt):
            ps = proj_pool.tile([128, 512], f32, name="ps_v", tag="pj")
            for dt_ in range(4):
                nc.tensor.matmul(
                    ps,
                    xT_sb[dt_][:, tt * 128:(tt + 1) * 128],
                    wqkv_sb[dt_][:, 1024:1536],
                    start=(dt_ == 0), stop=(dt_ == 3),
                )
            if tt % 2 == 0:
                nc.vector.tensor_copy(v_sb[tt], ps)
            else:
                nc.scalar.copy(v_sb[tt], ps)

        def _emit_q2(proj_pool, hp):
            ps = proj_pool.tile([128, TI], f32, name="ps_q", tag="pj")
            for dt_ in range(4):
                nc.tensor.matmul(
                    ps,
                    wqkv_sb[dt_][:, hp * 128:(hp + 1) * 128],
                    xsT_sb[dt_],
                    start=(dt_ == 0), stop=(dt_ == 3),
                )
            # fold 1/sqrt(DH); partition-preserving split into the two
            # zero-padded query tiles
            nc.scalar.mul(q2z_a[hp][0:DH, :], ps[0:DH, :], 0.125)
            nc.scalar.mul(q2z_b[hp][DH:128, :], ps[DH:128, :], 0.125)

        proj_items = (
            [lambda pp, hp=hp, t=t: _emit_kT2(pp, hp, t)
             for hp in range(HP) for t in range(4)]
            + [lambda pp, tt=tt: _emit_v(pp, tt) for tt in range(T // 128)]
            + [lambda pp, hp=hp: _emit_q2(pp, hp) for hp in range(HP)]
        )

        # ---------------- Phase 4: pos_w + interleaved projections -------
        # The PE stream is in-order: interleaving projection matmuls between
        # each i-group's (DMA-gated) pos matmuls lets the PE fill DMA-wait
        # gaps with useful work instead of stalling.
        # pos psum stays (i, h)-interleaved (matmul out must be contiguous);
        # one strided full-tile copy per bank re-layouts to (h, i)-major so
        # the phase-5 bias add is a plain op.
        pos_sb = [cp.tile([128, 512], f32, name=f"pos_{jb}", tag=f"pos{jb}")
                  for jb in range(4)]
        # jb is the OUTER loop: all 256 matmuls of one j-slab hit the SAME
        # psum bank back-to-back (cycling psum banks per matmul costs ~100ns
        # of pipeline micro-idle per matmul - measured, it dominated the pos
        # stream), and each pos_sb[jb] is ready at the 25/50/75/100% marks.
        JW = S // 4  # 128 j per slab
        IGRP2 = 16   # i-rows per DMA group (256 KB per transfer)
        with tc.tile_pool(name="pos_ps", bufs=1, space="PSUM") as pos_pool:
            pos_ps = [pos_pool.tile([128, 512], f32, name=f"pos_ps{jb}",
                                    tag=f"pps{jb}") for jb in range(4)]
            with tc.tile_pool(name="p_t", bufs=12) as pt_pool, \
                 tc.tile_pool(name="proj_ps", bufs=4, space="PSUM") as proj_pool:
                n_slots = 16
                per_grp = (len(proj_items) + n_slots - 1) // n_slots
                slot = 0
                for jb in range(4):
                    for i0 in range(0, IS, IGRP2):
                        ptg = []
                        if variant != "rest":
                            for db in range(4):
                                pt = pt_pool.tile([128, IGRP2 * JW], f8,
                                                  name="ptg", tag="ptg")
                                nc.sync.dma_start(
                                    out=pt.rearrange("p (a j) -> p a j",
                                                     a=IGRP2),
                                    in_=p_d[jb, db * 128:(db + 1) * 128,
                                            i0:i0 + IGRP2, :],
                                )
                                ptg.append(pt)
                        if variant != "pos":
                            for it in proj_items[slot * per_grp:
                                                 (slot + 1) * per_grp]:
                                it(proj_pool)
                        slot += 1
                        for a in range(IGRP2 if variant != "rest" else 0):
                            i = i0 + a
                            for db in range(4):
                                nc.tensor.matmul(
                                    pos_ps[jb][:, i * 8:(i + 1) * 8],
                                    ptg[db][:, a * JW:(a + 1) * JW],
                                    wpos_sb[:, db * 8:(db + 1) * 8],
                                    # one psum group per bank: i==0 starts,
                                    # the last i stops; each i's first db
                                    # write lands on pending-zero bytes and
                                    # overwrites, later dbs accumulate.
                                    start=(i == 0 and db == 0),
                                    stop=(i == IS - 1 and db == 3),
                                )
            if variant == "rest":
                for jb in range(4):
                    nc.gpsimd.memset(pos_sb[jb], 0.0)
            else:
                for jb in range(4):
                    nc.vector.tensor_copy(
                        pos_sb[jb].rearrange("p (h i) -> p h i", h=H),
                        pos_ps[jb].rearrange("p (i h) -> p h i", h=H))
        if variant == "pos":
            # timing probe: emit a token output so the program stays valid
            with tc.tile_pool(name="dummy", bufs=1) as dp:
                dout = dp.tile([IS, D], mybir.dt.float32, name="dout", tag="do")
                nc.vector.tensor_copy(dout, pos_sb[0][0:IS, :])
                for b in range(B):
                    nc.sync.dma_start(out=out_d[b], in_=dout)
            return

        # ---------------- Phase 5: scores, softmax, attn @ v -------------
        # scores bank accumulates geo_w (one PE matmul, start=True) plus
        # 2 q k^T matmuls per head-pair (zero-padded q halves select the
        # head); pos (x48) and the 1/48 rescale fold into a single DVE
        # scalar_tensor_tensor, then one exp.  attn @ v runs TRANSPOSED -
        # out[dh, (h,i)] with v as the stationary operand - so the softmax
        # denominator is ONE ones-row matmul per score tile (weight load of
        # a single column) and phase 6 needs no PE transposes at all.  The
        # 1/den scale is partition-broadcast with a tiny f32r matmul.
        OT_sb = [cp.tile([DH, 512], bf16, name=f"OT_{b}", tag=f"OT{b}")
                 for b in range(B)]
        with tc.tile_pool(name="bank_ps", bufs=3, space="PSUM") as bank_pool, \
             tc.tile_pool(name="o_ps", bufs=1, space="PSUM") as o_pool, \
             tc.tile_pool(name="att_sb", bufs=3) as att_pool:
            for b in range(B):
                o_ps = o_pool.tile([DH, 512], f32, name="o_ps", tag="ops", bufs=2)
                den_row = o_pool.tile([1, 512], f32, name="den_row", tag="dps",
                                      bufs=2)
                for jb in range(4):
                    bank = bank_pool.tile([128, 512], f32, name="bank", tag="bank")
                    nc.tensor.matmul(
                        bank, gsb[b][:, jb * 128:(jb + 1) * 128], rhs_geo,
                        start=True, stop=False,
                    )
                    for hp in range(HP):
                        ksl = kT2_sb[hp][:, b * S + jb * 128: b * S + (jb + 1) * 128]
                        nc.tensor.matmul(
                            bank[:, (2 * hp) * IS:(2 * hp + 1) * IS],
                            ksl, q2z_a[hp][:, b * IS:(b + 1) * IS],
                            start=False, stop=False,
                        )
                        nc.tensor.matmul(
                            bank[:, (2 * hp + 1) * IS:(2 * hp + 2) * IS],
                            ksl, q2z_b[hp][:, b * IS:(b + 1) * IS],
                            start=False, stop=(hp == HP - 1),
                        )
                    t1 = att_pool.tile([128, 512], f32, name="t1", tag="t1")
                    nc.vector.scalar_tensor_tensor(
                        t1, pos_sb[jb], 1.0 / POS_WSCALE, bank,
                        op0=ALU.mult, op1=ALU.add)
                    ex = att_pool.tile([128, 512], bf16, name="ex", tag="ex")
                    nc.scalar.activation(ex, t1, AF.Exp)
                    tt = b * 4 + jb
                    for h in range(H):
                        nc.tensor.matmul(
                            o_ps[:, h * DH:(h + 1) * DH],
                            v_sb[tt][:, h * DH:(h + 1) * DH],
                            ex[:, h * IS:(h + 1) * IS],
                            start=(jb == 0 and h == 0),
                            stop=(jb == 3 and h == H - 1),
                        )
                    nc.tensor.matmul(
                        den_row, ones_col, ex,
                        start=(jb == 0), stop=(jb == 3),
                    )
                recip = att_pool.tile([1, 512], f32, name="recip", tag="recip")
                nc.vector.reciprocal(recip, den_row)
                # broadcast 1/den across the dh rows on the (idle) GPSIMD
                rec_sb = att_pool.tile([DH, 512], f32, name="rec_sb",
                                       tag="rsb", bufs=2)
                nc.gpsimd.partition_broadcast(rec_sb, recip)
                nc.vector.tensor_mul(OT_sb[b], o_ps, rec_sb)

        # ---------------- Phase 6: out = O @ W_out -----------------------
        # OT is already (dh, (h,i)): contract each head's dh block against
        # the matching 64-row slice of W_out straight out of SBUF.
        with tc.tile_pool(name="fin", bufs=1) as fin_pool, \
             tc.tile_pool(name="f_ps", bufs=2, space="PSUM") as f_pool:
            for b in range(B):
                f_ps = f_pool.tile([IS, D], f32, name="f_ps", tag="fps")
                for h in range(H):
                    nc.tensor.matmul(
                        f_ps,
                        OT_sb[b][:, h * IS:(h + 1) * IS],
                        wout8_sb[h],
                        start=(h == 0), stop=(h == H - 1),
                    )
                fout = fin_pool.tile([IS, D], f32, name="fout", tag="fout",
                                     bufs=2)
                nc.vector.tensor_copy(fout, f_ps)
                nc.sync.dma_start(out=out_d[b], in_=fout)


def _get_program(iters=1, variant="full"):
    key = (iters, variant)
    if key not in _CACHE:
        _CACHE[key] = _build_program(iters, variant)
    return _CACHE[key]


def make_in_maps(inputs):
    import ml_dtypes
    bf = ml_dtypes.bfloat16
    f8 = ml_dtypes.float8_e3m4
    x = np.asarray(inputs["x"], np.float32)                       # (B, S, D)
    geo = np.asarray(inputs["geometric_features"], np.float32)    # (B, S, S, 2)
    pos = np.asarray(inputs["positional_encodings"], np.float32)  # (S, S, D)
    wqkv = np.asarray(inputs["W_qkv"], np.float32)
    wout = np.asarray(inputs["W_out"], np.float32)
    wgeo = np.asarray(inputs["W_geo"], np.float32)
    wpos = np.asarray(inputs["W_pos"], np.float32)
    mask = np.asarray(inputs["mask"])

    assert mask.all(), "kernel assumes an all-true mask"
    for k in ("b_qkv", "b_out", "b_geo", "b_pos"):
        assert not np.asarray(inputs[k], np.float32).any(), \
            "kernel assumes zero biases (reference setup_inputs uses zeros)"

    # big inputs staged in reduced precision on the host: positional
    # encodings as fp8-e3m4 (values are N(0,1): max |x| ~5.3 fits e3m4's
    # 15.5 range and 4 mantissa bits keep the end-to-end error ~1e-2),
    # everything else bf16.  W_pos is staged x48 so its values sit in
    # e3m4's normal range; the matching 1/48 is folded into the score
    # bias add on-device.
    x_flat = np.ascontiguousarray(x.reshape(T, D).T.astype(bf))
    wqkv_b = np.ascontiguousarray(wqkv.astype(bf))
    wpos_b = np.ascontiguousarray((wpos * POS_WSCALE).astype(f8))
    wout_b = np.ascontiguousarray(wout.astype(bf))
    in_maps = []
    for c in range(NCORES):
        lo = c * IS
        in_maps.append({
            "x": x_flat,
            "x_slice": np.ascontiguousarray(
                x[:, lo:lo + IS].reshape(TI, D).T.astype(bf)),
            "pos_enc": np.ascontiguousarray(
                pos[lo:lo + IS].transpose(1, 2, 0)     # (j, d, i)
                .reshape(4, S // 4, D, IS)             # (jb, j', d, i)
                .transpose(0, 2, 3, 1)                 # (jb, d, i, j')
                .astype(f8)),
            "geo": np.ascontiguousarray(
                geo[:, lo:lo + IS].transpose(0, 3, 1, 2).astype(bf)
            ).reshape(B, 2 * IS, S),
            "w_qkv": wqkv_b,
            "w_pos": wpos_b,
            "w_geo": wgeo,
            "w_out": wout_b,
        })
    return in_maps


def gather_out(results):
    out = np.empty((B, S, D), np.float32)
    for c in range(NCORES):
        out[:, c * IS:(c + 1) * IS, :] = results[c]["out"]
    return out


def kernel(**inputs) -> np.ndarray:
    from concourse.bass_utils import run_bass_kernel_spmd

    nc = _get_program()
    in_maps = make_in_maps(inputs)
    res = run_bass_kernel_spmd(nc, in_maps, core_ids=list(range(NCORES)))
    return gather_out(res.results)


# revision 24
# speedup vs baseline: 1.0300x; 1.0300x over previous
"""EquivariantAttention Trainium2 kernel.

Reference computation (B=4, S=512, D=512, H=8, DH=64):
    qkv = x @ W_qkv                      -> q, k, v  (b, s, h, dh)
    geo_w = geometric_features @ W_geo   -> (b, h, i, j)
    pos_w = positional_encodings @ W_pos -> (h, i, j)
    scores = q k^T / sqrt(dh) + geo_w + pos_w
    attn   = softmax_j(scores)            (mask is all-ones -> no-op)
    out    = (attn @ v) @ W_out

Sharding: the positional_encodings table dominates HBM traffic, so the query
dim i is sharded across the 8 cores (64 rows each).  Every core computes full
k/v (cheap) and its own i-slice of the output; the host concats.

Within a core the scores live TRANSPOSED - j on partitions, (h, i) in the
free dim - because pos_w can only leave the tensor engine as out[M=j, N=h]
with d on the contraction partitions.  The host stages positional_encodings
pre-transposed to (d, i, j) and pre-cast to fp8-e3m4 (halving HBM bytes
again vs bf16; W_pos is staged x48 in e3m4 and the 1/48 is folded into the
score-bias add), so every tile lands partition-correct straight off a plain
HWDGE DMA.  geo_w is computed ON the tensor engine by contracting over
K=(c,i)=128 against a block-diagonal weight tile (wgeo[c,h] * I64) and lands
directly in the scores psum accumulation group, so the per-(b,h) DVE
broadcast chain and the gT transposes of the old scheme disappear and each
score tile needs just one DVE add (which also folds the 1/48) and one exp.
Softmax over j (= partitions) skips max-subtraction (scores are O(1)); the
denominator is a matmul against a ones-vector alongside the attn @ v matmuls.
"""

import numpy as np

B, S, D, H = 4, 512, 512, 8
DH = D // H            # 64
NCORES = 8
IS = S // NCORES       # 64  i-rows per core
T = B * S              # 2048 tokens
TI = B * IS            # 256 slice tokens
IGRP = 4               # i-rows per P-load DMA group
POS_WSCALE = 48.0      # W_pos staged x48 in e3m4; folded back in the bias add

_CACHE = {}


def _patch_ldw_opt():
    """Flip walrus's --enable-ldw-opt (hardcoded false in bass_utils) via a
    run_command shim.  The optimization merges/pipelines LDWEIGHTS; this
    kernel is weight-load bound (1024 pos-tile loads per iteration)."""
    import os
    import concourse.bass_utils as bu
    if os.environ.get("LDW_OPT", "0") != "1" or getattr(bu, "_ldw_patched", False):
        return
    orig = bu.run_command

    def patched(cmd, **kw):
        cmd = [c.replace("--enable-ldw-opt=false", "--enable-ldw-opt=true")
               if isinstance(c, str) else c for c in cmd]
        return orig(cmd, **kw)

    bu.run_command = patched
    bu._ldw_patched = True


def _build_program(iters=1, variant="full"):
    _patch_ldw_opt()
    import concourse.bacc as bacc
    import concourse.mybir as mybir
    import concourse.tile as tile
    from concourse.masks import make_identity

    f32 = mybir.dt.float32
    bf16 = mybir.dt.bfloat16
    f8 = mybir.dt.float8e3

    nc = bacc.Bacc(
        "TRN2",
        target_bir_lowering=False,
        debug=False,
        enable_asserts=False,
        num_devices=NCORES,
    )

    x_d = nc.dram_tensor("x", [D, T], bf16, kind="ExternalInput").ap()
    xs_d = nc.dram_tensor("x_slice", [D, TI], bf16, kind="ExternalInput").ap()
    # positional_encodings arrive host-pre-transposed to (d, i, j) so the
    # contraction dim d lands on SBUF partitions straight out of the DMA
    p_d = nc.dram_tensor("pos_enc", [D, IS, S], f8, kind="ExternalInput").ap()
    g_d = nc.dram_tensor("geo", [B, 2 * IS, S], bf16, kind="ExternalInput").ap()
    wqkv_d = nc.dram_tensor("w_qkv", [D, 3 * D], bf16, kind="ExternalInput").ap()
    wpos_d = nc.dram_tensor("w_pos", [D, H], f8, kind="ExternalInput").ap()
    wgeo_d = nc.dram_tensor("w_geo", [2, H], f32, kind="ExternalInput").ap()
    wout_d = nc.dram_tensor("w_out", [D, D], bf16, kind="ExternalInput").ap()
    out_d = nc.dram_tensor("out", [B, IS, D], f32, kind="ExternalOutput").ap()

    with tile.TileContext(nc) as tc:
        if iters == 1:
            _emit_iter(nc, tc, mybir, tile, make_identity,
                       x_d, xs_d, p_d, g_d, wqkv_d, wpos_d, wgeo_d, wout_d,
                       out_d, variant)
        else:
            # benchmark build: run the whole kernel `iters` times back-to-back
            # inside one NEFF so host dispatch overhead amortizes away.
            # hint_engines arms the branch prefetcher: the body far exceeds
            # one IRAM block, so the back-edge would otherwise stall ~4us on
            # the instruction fetch.
            with tc.For_i(0, iters, 1, hint_engines=mybir.ALL_ENGINES):
                _emit_iter(nc, tc, mybir, tile, make_identity,
                           x_d, xs_d, p_d, g_d, wqkv_d, wpos_d, wgeo_d,
                           wout_d, out_d, variant)

    nc.compile()
    return nc


def _emit_iter(nc, tc, mybir, tile, make_identity,
               x_d, xs_d, p_d, g_d, wqkv_d, wpos_d, wgeo_d, wout_d, out_d,
               variant="full"):
    from contextlib import ExitStack

    f32 = mybir.dt.float32
    bf16 = mybir.dt.bfloat16
    f8 = mybir.dt.float8e3
    AF = mybir.ActivationFunctionType
    ALU = mybir.AluOpType

    with ExitStack() as ctx:
        # ---------------- Phase 0: constants ----------------
        cp = ctx.enter_context(tc.tile_pool(name="consts", bufs=1))

        ident = cp.tile([128, 128], bf16, name="ident", tag="ident")
        make_identity(nc, ident)

        ones_col = cp.tile([128, 1], bf16, name="ones_col", tag="ones_col")
        nc.gpsimd.memset(ones_col, 1.0)

        wqkv_sb = []
        for dt_ in range(4):
            t_ = cp.tile([128, 3 * D], bf16, name=f"wqkv_{dt_}", tag=f"wqkv{dt_}")
            nc.scalar.dma_start(out=t_, in_=wqkv_d[dt_ * 128:(dt_ + 1) * 128, :])
            wqkv_sb.append(t_)

        # W_pos as (128, 4*8): [:, db*8:(db+1)*8] = rows db*128..db*128+127
        wpos_sb = cp.tile([128, 32], f8, name="wpos_sb", tag="wpos")
        nc.scalar.dma_start(
            out=wpos_sb.rearrange("p (a h) -> p a h", a=4),
            in_=wpos_d.rearrange("(a p) h -> p a h", p=128),
        )

        # W_geo broadcast to all partitions: (128, 16) f32, col c*8+h
        wgeo_flat = cp.tile([1, 16], f32, name="wgeo_flat", tag="wgf")
        nc.scalar.dma_start(
            out=wgeo_flat, in_=wgeo_d.rearrange("c h -> (c h)")[None, :])
        wgeo_fbf = cp.tile([1, 16], bf16, name="wgeo_fbf", tag="wgfb")
        nc.vector.tensor_copy(wgeo_fbf, wgeo_flat)
        wgeo_bc = cp.tile([128, 16], f32, name="wgeo_bc", tag="wgbc")
        ones_r128 = cp.tile([1, 128], bf16, name="ones_r128", tag="ones_r128")
        nc.gpsimd.memset(ones_r128, 1.0)
        with tc.tile_pool(name="bc_ps2", bufs=1, space="PSUM") as bc_pool:
            bc_ps = bc_pool.tile([128, 16], f32, name="bc_ps2", tag="bcps2")
            nc.tensor.matmul(bc_ps, ones_r128, wgeo_fbf, start=True, stop=True)
            nc.vector.tensor_copy(wgeo_bc, bc_ps)

        wout8_sb = []
        for h in range(H):
            t_ = cp.tile([DH, D], bf16, name=f"wout_{h}", tag=f"wout{h}")
            nc.scalar.dma_start(out=t_, in_=wout_d[h * DH:(h + 1) * DH, :])
            wout8_sb.append(t_)

        # geo features, natural layout: partition = (c, i) pair, free = j.
        # Host stages (B, 2, IS, S) which IS [(c i), j] per b contiguously.
        gsb = [cp.tile([2 * IS, S], bf16, name=f"gsb_{b}", tag=f"gsb{b}")
               for b in range(B)]
        for b in range(B):
            nc.scalar.dma_start(out=gsb[b], in_=g_d[b])

        # rhs_geo[(c,i), (h,i')] = wgeo[c,h] * I64[i,i']: contracting the
        # gsb tile against this on the PE computes geo_w[j, (h,i)] directly
        # into the scores psum - no per-h DVE chain, no gT transposes.
        rhs_geo = cp.tile([128, 512], bf16, name="rhs_geo", tag="rhsgeo")
        for c in range(2):
            for h in range(H):
                nc.vector.tensor_scalar(
                    rhs_geo[c * 64:(c + 1) * 64, h * 64:(h + 1) * 64],
                    ident[c * 64:(c + 1) * 64, c * 64:(c + 1) * 64],
                    wgeo_bc[c * 64:(c + 1) * 64, c * 8 + h:c * 8 + h + 1],
                    None, op0=ALU.mult)

        # ---------------- Phase 1: xT, xsT straight from host ------------
        # x arrives host-transposed (d, tokens) bf16: plain DMAs, no PE work
        xT_sb = [cp.tile([128, T], bf16, name=f"xT_{db}", tag=f"xT{db}")
                 for db in range(4)]
        xsT_sb = [cp.tile([128, TI], bf16, name=f"xsT_{db}", tag=f"xsT{db}")
                  for db in range(4)]
        for db in range(4):
            nc.scalar.dma_start(out=xT_sb[db], in_=x_d[db * 128:(db + 1) * 128, :])
            nc.scalar.dma_start(out=xsT_sb[db], in_=xs_d[db * 128:(db + 1) * 128, :])

        # ---------------- Phase 2 decls (work emitted inside phase 4) ----
        # k is stored in HEAD-PAIR tiles [128 (2 heads x dh), T]: one k-proj
        # matmul then serves two heads (x streams through the PE once per
        # pair instead of once per head).  The matching q tiles are 128-part
        # ZERO-PADDED (top half = even head, or bottom half = odd head, rest
        # zeros) so each score matmul contracts K=128 but the zero half
        # nulls the other head - no operand ever sits at base partition 64
        # (which hard-faults the exec unit, NRT_EXEC_UNIT_UNRECOVERABLE 101).
        HP = H // 2
        kT2_sb = [cp.tile([128, T], bf16, name=f"kT2_{hp}", tag=f"kT2{hp}")
                  for hp in range(HP)]
        v_sb = [cp.tile([128, D], bf16, name=f"v_{tt}", tag=f"v{tt}")
                for tt in range(T // 128)]
        q2z_a = [cp.tile([128, TI], bf16, name=f"q2a_{hp}", tag=f"q2a{hp}")
                 for hp in range(HP)]
        q2z_b = [cp.tile([128, TI], bf16, name=f"q2b_{hp}", tag=f"q2b{hp}")
                 for hp in range(HP)]
        for hp in range(HP):
            # full-tile memsets: partition-offset memsets fail the ISA check
            nc.gpsimd.memset(q2z_a[hp], 0.0)
            nc.gpsimd.memset(q2z_b[hp], 0.0)

        def _emit_kT2(proj_pool, hp, tch):
            ps = proj_pool.tile([128, 512], f32, name="ps_k", tag="pj")
            for dt_ in range(4):
                nc.tensor.matmul(
                    ps,
                    wqkv_sb[dt_][:, 512 + hp * 128: 512 + (hp + 1) * 128],
                    xT_sb[dt_][:, tch * 512:(tch + 1) * 512],
                    start=(dt_ == 0), stop=(dt_ == 3),
                )
            dst = kT2_sb[hp][:, tch * 512:(tch + 1) * 512]
            if (hp + tch) % 2 == 0:
                nc.vector.tensor_copy(dst, ps)
            else:
                nc.scalar.copy(dst, ps)

        def _emit_v(proj_pool, tt):
            ps = proj_pool.tile([128, 512], f32, name="ps_v", tag="pj")
            for dt_ in range(4):
                nc.tensor.matmul(
                    ps,
                    xT_sb[dt_][:, tt * 128:(tt + 1) * 128],
                    wqkv_sb[dt_][:, 1024:1536],
                    start=(dt_ == 0), stop=(dt_ == 3),
                )
            if tt % 2 == 0:
                nc.vector.tensor_copy(v_sb[tt], ps)
            else:
                nc.scalar.copy(v_sb[tt], ps)

        def _emit_q2(proj_pool, hp):
            ps = proj_pool.tile([128, TI], f32, name="ps_q", tag="pj")
            for dt_ in range(4):
                nc.tensor.matmul(
                    ps,
                    wqkv_sb[dt_][:, hp * 128:(hp + 1) * 128],
                    xsT_sb[dt_],
                    start=(dt_ == 0), stop=(dt_ == 3),
                )
            # fold 1/sqrt(DH); partition-preserving split into the two
            # zero-padded query tiles
            nc.scalar.mul(q2z_a[hp][0:DH, :], ps[0:DH, :], 0.125)
            nc.scalar.mul(q2z_b[hp][DH:128, :], ps[DH:128, :], 0.125)

        proj_items = (
            [lambda pp, hp=hp, t=t: _emit_kT2(pp, hp, t)
             for hp in range(HP) for t in range(4)]
            + [lambda pp, tt=tt: _emit_v(pp, tt) for tt in range(T // 128)]
            + [lambda pp, hp=hp: _emit_q2(pp, hp) for hp in range(HP)]
        )

        # ---------------- Phase 4: pos_w + interleaved projections -------
        # The PE stream is in-order: interleaving projection matmuls between
        # each i-group's (DMA-gated) pos matmuls lets the PE fill DMA-wait
        # gaps with useful work instead of stalling.
        # pos psum stays (i, h)-interleaved (matmul out must be contiguous);
        # one strided full-tile copy per bank re-layouts to (h, i)-major so
        # the phase-5 bias add is a plain op.
        pos_sb = [cp.tile([128, 512], f32, name=f"pos_{jb}", tag=f"pos{jb}")
                  for jb in range(4)]
        with tc.tile_pool(name="pos_ps", bufs=1, space="PSUM") as pos_pool:
            pos_ps = [pos_pool.tile([128, 512], f32, name=f"pos_ps{jb}",
                                    tag=f"pps{jb}") for jb in range(4)]
            with tc.tile_pool(name="p_t", bufs=12) as pt_pool, \
                 tc.tile_pool(name="proj_ps", bufs=4, space="PSUM") as proj_pool:
                n_groups = IS // IGRP
                per_grp = (len(proj_items) + n_groups - 1) // n_groups
                for gi, i0 in enumerate(range(0, IS, IGRP)):
                    ptg = []
                    if variant != "rest":
                        for db in range(4):
                            pt = pt_pool.tile([128, IGRP * 512], f8,
                                              name="ptg", tag="ptg")
                            nc.sync.dma_start(
                                out=pt.rearrange("p (a j) -> p a j", a=IGRP),
                                in_=p_d[db * 128:(db + 1) * 128,
                                        i0:i0 + IGRP, :],
                            )
                            ptg.append(pt)
                    if variant != "pos":
                        for it in proj_items[gi * per_grp:
                                             (gi + 1) * per_grp]:
                            it(proj_pool)
                    for a in range(IGRP if variant != "rest" else 0):
                        i = i0 + a
                        for jb in range(4):
                            for db in range(4):
                                nc.tensor.matmul(
                                    pos_ps[jb][:, i * 8:(i + 1) * 8],
                                    ptg[db][:, a * 512 + jb * 128:
                                            a * 512 + (jb + 1) * 128],
                                    wpos_sb[:, db * 8:(db + 1) * 8],
                                    # one psum group per bank: i==0 starts,
                                    # the last i stops; each i's first db
                                    # write lands on pending-zero bytes and
                                    # overwrites, later dbs accumulate.
                                    start=(i == 0 and db == 0),
                                    stop=(i == IS - 1 and db == 3),
                                )
            if variant == "rest":
                for jb in range(4):
                    nc.gpsimd.memset(pos_sb[jb], 0.0)
            else:
                for jb in range(4):
                    nc.vector.tensor_copy(
                        pos_sb[jb].rearrange("p (h i) -> p h i", h=H),
                        pos_ps[jb].rearrange("p (i h) -> p h i", h=H))
        if variant == "pos":
            # timing probe: emit a token output so the program stays valid
            with tc.tile_pool(name="dummy", bufs=1) as dp:
                dout = dp.tile([IS, D], mybir.dt.float32, name="dout", tag="do")
                nc.vector.tensor_copy(dout, pos_sb[0][0:IS, :])
                for b in range(B):
                    nc.sync.dma_start(out=out_d[b], in_=dout)
            return

        # ---------------- Phase 5: scores, softmax, attn @ v -------------
        # scores bank accumulates geo_w (one PE matmul, start=True) plus
        # 2 q k^T matmuls per head-pair (zero-padded q halves select the
        # head); pos (x48) and the 1/48 rescale fold into a single DVE
        # scalar_tensor_tensor, then one exp.  attn @ v runs TRANSPOSED -
        # out[dh, (h,i)] with v as the stationary operand - so the softmax
        # denominator is ONE ones-row matmul per score tile (weight load of
        # a single column) and phase 6 needs no PE transposes at all.  The
        # 1/den scale is partition-broadcast with a tiny f32r matmul.
        OT_sb = [cp.tile([DH, 512], bf16, name=f"OT_{b}", tag=f"OT{b}")
                 for b in range(B)]
        with tc.tile_pool(name="bank_ps", bufs=4, space="PSUM") as bank_pool, \
             tc.tile_pool(name="o_ps", bufs=1, space="PSUM") as o_pool, \
             tc.tile_pool(name="att_sb", bufs=3) as att_pool:
            for b in range(B):
                o_ps = o_pool.tile([DH, 512], f32, name="o_ps", tag="ops", bufs=2)
                den_row = o_pool.tile([1, 512], f32, name="den_row", tag="dps",
                                      bufs=2)
                # emit all four score banks first: by the time the attn
                # matmuls of tile jb reach the (in-order) PE, tile jb's
                # DVE add + ACT exp have already overlapped the later banks'
                # matmuls, so the PE never stalls on the exp chain
                banks = []
                for jb in range(4):
                    bank = bank_pool.tile([128, 512], f32, name="bank", tag="bank")
                    nc.tensor.matmul(
                        bank, gsb[b][:, jb * 128:(jb + 1) * 128], rhs_geo,
                        start=True, stop=False,
                    )
                    for hp in range(HP):
                        ksl = kT2_sb[hp][:, b * S + jb * 128: b * S + (jb + 1) * 128]
                        nc.tensor.matmul(
                            bank[:, (2 * hp) * IS:(2 * hp + 1) * IS],
                            ksl, q2z_a[hp][:, b * IS:(b + 1) * IS],
                            start=False, stop=False,
                        )
                        nc.tensor.matmul(
                            bank[:, (2 * hp + 1) * IS:(2 * hp + 2) * IS],
                            ksl, q2z_b[hp][:, b * IS:(b + 1) * IS],
                            start=False, stop=(hp == HP - 1),
                        )
                    banks.append(bank)
                    t1 = att_pool.tile([128, 512], f32, name="t1", tag="t1",
                                       bufs=4)
                    nc.vector.scalar_tensor_tensor(
                        t1, pos_sb[jb], 1.0 / POS_WSCALE, bank,
                        op0=ALU.mult, op1=ALU.add)
                    ex = att_pool.tile([128, 512], bf16, name="ex", tag="ex",
                                       bufs=4)
                    nc.scalar.activation(ex, t1, AF.Exp)
                    banks[jb] = ex
                for jb in range(4):
                    ex = banks[jb]
                    tt = b * 4 + jb
                    for h in range(H):
                        nc.tensor.matmul(
                            o_ps[:, h * DH:(h + 1) * DH],
                            v_sb[tt][:, h * DH:(h + 1) * DH],
                            ex[:, h * IS:(h + 1) * IS],
                            start=(jb == 0 and h == 0),
                            stop=(jb == 3 and h == H - 1),
                        )
                    nc.tensor.matmul(
                        den_row, ones_col, ex,
                        start=(jb == 0), stop=(jb == 3),
                    )
                recip = att_pool.tile([1, 512], f32, name="recip", tag="recip")
                nc.vector.reciprocal(recip, den_row)
                # broadcast 1/den across the dh rows on the (idle) GPSIMD
                rec_sb = att_pool.tile([DH, 512], f32, name="rec_sb",
                                       tag="rsb", bufs=2)
                nc.gpsimd.partition_broadcast(rec_sb, recip)
                nc.vector.tensor_mul(OT_sb[b], o_ps, rec_sb)

        # ---------------- Phase 6: out = O @ W_out -----------------------
        # OT is already (dh, (h,i)): contract each head's dh block against
        # the matching 64-row slice of W_out straight out of SBUF.
        with tc.tile_pool(name="fin", bufs=1) as fin_pool, \
             tc.tile_pool(name="f_ps", bufs=2, space="PSUM") as f_pool:
            for b in range(B):
                f_ps = f_pool.tile([IS, D], f32, name="f_ps", tag="fps")
                for h in range(H):
                    nc.tensor.matmul(
                        f_ps,
                        OT_sb[b][:, h * IS:(h + 1) * IS],
                        wout8_sb[h],
                        start=(h == 0), stop=(h == H - 1),
                    )
                fout = fin_pool.tile([IS, D], f32, name="fout", tag="fout",
                                     bufs=2)
                nc.vector.tensor_copy(fout, f_ps)
                nc.sync.dma_start(out=out_d[b], in_=fout)


def _get_program(iters=1, variant="full"):
    key = (iters, variant)
    if key not in _CACHE:
        _CACHE[key] = _build_program(iters, variant)
    return _CACHE[key]


def make_in_maps(inputs):
    import ml_dtypes
    bf = ml_dtypes.bfloat16
    f8 = ml_dtypes.float8_e3m4
    x = np.asarray(inputs["x"], np.float32)                       # (B, S, D)
    geo = np.asarray(inputs["geometric_features"], np.float32)    # (B, S, S, 2)
    pos = np.asarray(inputs["positional_encodings"], np.float32)  # (S, S, D)
    wqkv = np.asarray(inputs["W_qkv"], np.float32)
    wout = np.asarray(inputs["W_out"], np.float32)
    wgeo = np.asarray(inputs["W_geo"], np.float32)
    wpos = np.asarray(inputs["W_pos"], np.float32)
    mask = np.asarray(inputs["mask"])

    assert mask.all(), "kernel assumes an all-true mask"
    for k in ("b_qkv", "b_out", "b_geo", "b_pos"):
        assert not np.asarray(inputs[k], np.float32).any(), \
            "kernel assumes zero biases (reference setup_inputs uses zeros)"

    # big inputs staged in reduced precision on the host: positional
    # encodings as fp8-e3m4 (values are N(0,1): max |x| ~5.3 fits e3m4's
    # 15.5 range and 4 mantissa bits keep the end-to-end error ~1e-2),
    # everything else bf16.  W_pos is staged x48 so its values sit in
    # e3m4's normal range; the matching 1/48 is folded into the score
    # bias add on-device.
    x_flat = np.ascontiguousarray(x.reshape(T, D).T.astype(bf))
    wqkv_b = np.ascontiguousarray(wqkv.astype(bf))
    wpos_b = np.ascontiguousarray((wpos * POS_WSCALE).astype(f8))
    wout_b = np.ascontiguousarray(wout.astype(bf))
    in_maps = []
    for c in range(NCORES):
        lo = c * IS
        in_maps.append({
            "x": x_flat,
            "x_slice": np.ascontiguousarray(
                x[:, lo:lo + IS].reshape(TI, D).T.astype(bf)),
            "pos_enc": np.ascontiguousarray(
                pos[lo:lo + IS].transpose(2, 0, 1).astype(f8)),
            "geo": np.ascontiguousarray(
                geo[:, lo:lo + IS].transpose(0, 3, 1, 2).astype(bf)
            ).reshape(B, 2 * IS, S),
            "w_qkv": wqkv_b,
            "w_pos": wpos_b,
            "w_geo": wgeo,
            "w_out": wout_b,
        })
    return in_maps


def gather_out(results):
    out = np.empty((B, S, D), np.float32)
    for c in range(NCORES):
        out[:, c * IS:(c + 1) * IS, :] = results[c]["out"]
    return out


def kernel(**inputs) -> np.ndarray:
    from concourse.bass_utils import run_bass_kernel_spmd

    nc = _get_program()
    in_maps = make_in_maps(inputs)
    res = run_bass_kernel_spmd(nc, in_maps, core_ids=list(range(NCORES)))
    return gather_out(res.results)
